# revision 9
# baseline (speedup 1.0000x reference)
"""BQuantConv1d Trainium2 kernel.

Math: the reference's per-token LUT + gather is algebraically a matmul:
  out[n, f] = sum_i x[n, i] * W[i, f] + bias[f]
  W[8g+j, f] = sum_b scale[b, f] * (2*bit_{7-j}(binary[b, g, f]) - 1)

Sharding: 2 token-groups x 4 f-groups over 8 cores, no collectives
(host slices inputs / concatenates outputs; layout-only host work).
Contraction order is permuted to i' = j*128 + g (host permutes xT rows to
match) so each decoded weight chunk j lands on contiguous partitions.

Per core:
  - decode W'(1024, 256) from int16 codes with a sign-bit trick:
    W element = +-scale[b, f] exactly, built by XORing the bf16 scale's
    sign bit (scales arrive sign-pre-flipped) with the masked quant bit
    (c << (8+j)) & 0x8000, as int32 SWAR on DVE (bitvec ops are DVE-only,
    32-bit only); the b-reduction add tree is split: first level (h1) on
    DVE, lower levels (h2, w) on GPSIMD -- except the last chunk's h2/w,
    which stay on DVE so the tail does not pay the GPSIMD hand-off lag;
  - outT[f_shard, n_shard] = W'.T @ xT on the PE in bf16, accumulating
    the 8 contraction chunks across 8 concurrent PSUM banks (f32),
    W-chunk-outer so the PE consumes each chunk as it is decoded; the PE
    p-state ramp (0.65 -> 2.4 GHz after ~3us of busy) is pre-paid with a
    few throwaway matmuls on zeroed tiles while the decode warms up;
  - PSUM evacuation fuses the bias add (per-partition scalar, since the
    PSUM partition dim is f): Activation engine activation(Identity,
    bias=...), DVE / GPSIMD tensor_scalar(add); the 8 evacuations are
    spread across those three engines so the post-last-matmul tail is
    short; output DMAs are spread over the sync/scalar/vector queues.
"""

import numpy as np

try:
    import concourse.bass as bass  # noqa: F401
except ImportError:
    import sys

    sys.path.insert(0, "/opt/trn_rl_repo")
    import concourse.bass as bass  # noqa: F401

import concourse.bacc as bacc
import concourse.mybir as mybir
import concourse.tile as tile

B, T, NX, NF = 2, 2048, 1024, 1024
N_TOK = B * T
BITS = 8
G = NX // 8  # 128 code groups
PT, PF = 2, 4  # token-parallel x feature-parallel
TOK = N_TOK // PT  # tokens per core
NFS = NF // PF  # output features per core
P = 128
MM_N = 512  # moving free dim per matmul

AX = mybir.AxisListType
OP = mybir.AluOpType
F32 = mybir.dt.float32
BF16 = mybir.dt.float16  # compute dtype (fp16: same SWAR, more mantissa)
I16 = mybir.dt.int16
I32 = mybir.dt.int32
ACT_F = mybir.ActivationFunctionType
BF16NP = np.float16

MSK = -2147450880  # 0x80008000 as int32

# evacuation engine per PSUM group k = fb*nch + ch (see build_graph);
# GPSIMD cannot read PSUM, so only Act and DVE evacuate
EVAC_ENG = ["act", "vec", "act", "vec", "act", "vec", "act", "vec"]


def build_graph(nc, tok=TOK, nfs=NFS, pair_groups=None):
    del pair_groups  # legacy arg kept for test harness compatibility
    nfb = nfs // P  # f blocks of 128
    nch = tok // MM_N  # moving chunks
    xt_d = nc.dram_tensor("xt", (8, P, tok), BF16, kind="ExternalInput")
    cd_d = nc.dram_tensor("codes", (P, 8 * nfs), I16, kind="ExternalInput")
    sc_d = nc.dram_tensor("scales", (P, 8 * nfs), BF16, kind="ExternalInput")
    bi_d = nc.dram_tensor("biasv", (P, nfb), F32, kind="ExternalInput")
    out_d = nc.dram_tensor("out", (nfb, nch, P, MM_N), BF16, kind="ExternalOutput")

    with tile.TileContext(nc) as tc:
        with (
            tc.tile_pool(name="xp", bufs=8) as xp,
            tc.tile_pool(name="cp", bufs=8) as cp,
            tc.tile_pool(name="wp", bufs=8) as wp,
            tc.tile_pool(name="sgp", bufs=2) as sgp,
            tc.tile_pool(name="wsgp", bufs=1) as wsgp,
            tc.tile_pool(name="qp", bufs=4) as qp,
            tc.tile_pool(name="cst", bufs=1) as cst,
            tc.tile_pool(name="op", bufs=8) as op_,
            tc.tile_pool(name="pp", bufs=8, space="PSUM") as pp,
        ):
            # --- loads; codes first: decode is the critical path ---
            cd = cp.tile([P, 8 * nfs], I16, tag="cd")
            nc.sync.dma_start(cd[:], cd_d[:])
            sc_bc = cst.tile([P, 8 * nfs], BF16, tag="sc_bc")
            nc.sync.dma_start(sc_bc[:], sc_d[:])
            bi_t = cst.tile([P, nfb], F32, tag="bi_t")
            nc.sync.dma_start(bi_t[:], bi_d[:])
            xts = []
            for j in range(8):
                xt = xp.tile([P, tok], BF16, tag="xt")
                nc.sync.dma_start(xt[:], xt_d[j])
                xts.append(xt)

            # --- Activation table prime + PE p-state warmup fodder ---
            prime = cst.tile([P, 1], BF16, tag="prime")
            nc.scalar.activation(
                prime[:], bi_t[:, 0:1], ACT_F.Identity, bias=bi_t[:, 0:1]
            )
            wz = cst.tile([P, P], BF16, tag="wz")
            nc.gpsimd.memset(wz[:], 0.0)
            xz = cst.tile([P, MM_N], BF16, tag="xz")
            nc.gpsimd.memset(xz[:], 0.0)

            pss = {}
            for fb in range(nfb):
                for ch in range(nch):
                    pss[(fb, ch)] = pp.tile(
                        [P, MM_N], F32, tag="ps", name=f"ps{fb}_{ch}"
                    )
            # Throwaway matmuls: ramp the PE to full clock while the DVE
            # decode runs, and keep the busy streak alive until the first
            # real chunk lands (idle gaps reset the p-state ramp); the real
            # j=0 matmul restarts its bank (start=True).
            for _ in range(20):
                nc.tensor.matmul(pss[(0, 0)][:], wz[:], xz[:], start=True, stop=False)

            # --- decode W chunks ---
            # Sign-bit trick: masked quant bit (inverted) XORed onto the
            # bf16 scale's sign gives +-scale exactly.  Bit ops run as int32
            # SWAR over int16-lane pairs; the bit inversion is folded into a
            # one-time sign-flip of the scale tile (host pre-negates):
            #   ((c << (8+j)) & M) ^ (sc ^ M)  ==  ((~c << (8+j)) & M) ^ sc
            sgs, wsgs, h1s, ws = [], [], [], []

            def emit_tsp(j):
                sg = sgp.tile([P, 8 * nfs], I16, tag="sg", name=f"sg{j}")
                nc.vector.tensor_scalar(
                    sg[:].bitcast(I32), cd[:].bitcast(I32), 8 + j, MSK,
                    OP.logical_shift_left, OP.bitwise_and,
                )
                sgs.append(sg)

            def emit_xor_h1(j):
                wsg = wsgp.tile([P, 8 * nfs], I16, tag="wsg", name=f"wsg{j}")
                nc.vector.tensor_tensor(
                    wsg[:].bitcast(I32), sgs[j][:].bitcast(I32),
                    sc_bc[:].bitcast(I32), OP.bitwise_xor,
                )
                wsgs.append(wsg)
                wv = wsg[:].bitcast(BF16)
                h1 = qp.tile([P, 4 * nfs], BF16, tag="h1", name=f"h1_{j}")
                nc.vector.tensor_tensor(
                    h1[:], wv[:, : 4 * nfs], wv[:, 4 * nfs :], OP.add
                )
                h1s.append(h1)

            def emit_h2w(j, eng):
                h2 = qp.tile([P, 2 * nfs], BF16, tag="h2", name=f"h2_{j}")
                eng.tensor_tensor(
                    h2[:], h1s[j][:, : 2 * nfs], h1s[j][:, 2 * nfs :], OP.add
                )
                w = wp.tile([P, nfs], BF16, tag="w", name=f"w{j}")
                eng.tensor_tensor(w[:], h2[:, :nfs], h2[:, nfs:], OP.add)
                ws.append(w)

            # DVE order: two TSPs up front to hide the scales-DMA wait,
            # then steady-state xor+h1 with the next TSP slotted between.
            emit_tsp(0)
            emit_tsp(1)
            emit_xor_h1(0)
            for j in range(1, 8):
                if j + 1 < 8:
                    emit_tsp(j + 1)
                emit_xor_h1(j)
            # gpsimd consumes h1_j for j=0..6; the last chunk's h2/w stay on
            # DVE so W7 lands without the cross-engine hand-off lag.
            for j in range(7):
                emit_h2w(j, nc.gpsimd)
            emit_h2w(7, nc.vector)
            # Python emission order above != engine order; Tile serializes
            # per engine in emission order, so reorder: we emitted all DVE
            # decode first, then gpsimd h2/w -- gpsimd only depends on h1_j,
            # which lands early; fine.

            # --- matmul: outT[f, n] = sum_j W_j.T @ xT_j  (bias in evac) ---
            groups = [(fb, ch) for fb in range(nfb) for ch in range(nch)]
            obs = {}
            for j in range(8):
                for k, (fb, ch) in enumerate(groups):
                    nc.tensor.matmul(
                        pss[(fb, ch)][:],
                        ws[j][:, fb * P : (fb + 1) * P],
                        xts[j][:, ch * MM_N : (ch + 1) * MM_N],
                        start=(j == 0),
                        stop=(j == 7),
                    )
                    if j == 7:
                        # evacuate with fused bias add; Act/DVE alternate so
                        # the tail pipeline drains in parallel
                        ob = op_.tile([P, MM_N], BF16, tag="ob", name=f"ob{k}")
                        obs[k] = (ob, fb, ch)
                        bcol = bi_t[:, fb : fb + 1]
                        if EVAC_ENG[k] == "act":
                            nc.scalar.activation(
                                ob[:], pss[(fb, ch)][:], ACT_F.Identity,
                                bias=bcol, scale=1.0,
                            )
                        else:
                            nc.vector.tensor_scalar(
                                ob[:], pss[(fb, ch)][:], bcol, None, OP.add
                            )
            # Output DMAs last, split over the SP and Act HWDGE queues.
            # Act-queue DMAs are emitted after every Act evacuation so a
            # DMA waiting on a DVE-evacuated tile never blocks an Act
            # evacuation behind it in the in-order SEQ stream.
            for k in sorted(obs):
                ob, fb, ch = obs[k]
                q = nc.sync if EVAC_ENG[k] == "act" else nc.scalar
                q.dma_start(out_d[fb, ch], ob[:])
    nc.compile()
    return nc


_I_PERM = 8 * (np.arange(NX) % G) + np.arange(NX) // G  # i' -> i


def host_prep(x, binary, scale, bias):
    """Layout-only sharding (plus x's bf16 compute-precision cast).
    Returns in_maps for cores 0..7 (pt = c//PF, pf = c%PF)."""
    x2 = np.ascontiguousarray(x.reshape(N_TOK, NX).T)[_I_PERM]  # (NX, N)
    x2 = x2.astype(BF16NP)  # compute dtype
    binary16 = binary.astype(np.int16)  # lossless: codes are 0..255
    in_maps = []
    for c in range(8):
        pt, pf = c // PF, c % PF
        f0 = pf * NFS
        xs = np.ascontiguousarray(x2[:, pt * TOK : (pt + 1) * TOK]).reshape(
            8, P, TOK
        )
        cs = np.ascontiguousarray(
            binary16[:, :, f0 : f0 + NFS].transpose(1, 0, 2)
        ).reshape(P, 8 * NFS)
        ss = np.ascontiguousarray(
            np.broadcast_to(
                (-scale[:, f0 : f0 + NFS].astype(BF16NP)).reshape(1, 8 * NFS),
                (P, 8 * NFS),
            )
        )
        bs = np.ascontiguousarray(
            bias[f0 : f0 + NFS].astype(np.float32).reshape(NFS // P, P).T
        )
        in_maps.append({"xt": xs, "codes": cs, "scales": ss, "biasv": bs})
    return in_maps


def host_assemble(results):
    """results[c]["out"]: (NFB, NCH, 128, MM_N) -> full (B, T, NF)."""
    outT = np.empty((NF, N_TOK), dtype=np.float32)
    for c in range(8):
        pt, pf = c // PF, c % PF
        o = np.asarray(results[c]["out"], dtype=np.float32)
        o = o.transpose(0, 2, 1, 3).reshape(NFS, TOK)
        outT[pf * NFS : (pf + 1) * NFS, pt * TOK : (pt + 1) * TOK] = o
    return np.ascontiguousarray(outT.T).reshape(B, T, NF)


_NC_CACHE = {}


def _get_nc():
    if "nc" not in _NC_CACHE:
        nc = bacc.Bacc(None, target_bir_lowering=False)
        build_graph(nc)
        _NC_CACHE["nc"] = nc
    return _NC_CACHE["nc"]


def kernel(**inputs):
    from concourse.bass_utils import run_bass_kernel_spmd

    inputs = {k: np.asarray(v) for k, v in inputs.items()}
    in_maps = host_prep(
        inputs["x"], inputs["binary"], inputs["scale"], inputs["bias"]
    )
    res = run_bass_kernel_spmd(_get_nc(), in_maps, core_ids=list(range(8)))
    return host_assemble(res.results)


# revision 14
# speedup vs baseline: 1.0646x; 1.0646x over previous
"""BQuantConv1d Trainium2 kernel.

Math: the reference's per-token LUT + gather is algebraically a matmul:
  out[n, f] = sum_i x[n, i] * W[i, f] + bias[f]
  W[8g+j, f] = sum_b scale[b, f] * (2*bit_{7-j}(binary[b, g, f]) - 1)

Sharding: 2 token-groups x 4 f-groups over 8 cores, no collectives
(host slices inputs / concatenates outputs; layout-only host work).
Contraction order is permuted to i' = j*128 + g (host permutes xT rows to
match) so each decoded weight chunk j lands on contiguous partitions.

Per core:
  - decode W'(1024, 256) from int16 codes with a sign-bit trick:
    W element = +-scale[b, f] exactly, built by XORing the bf16 scale's
    sign bit (scales arrive sign-pre-flipped) with the masked quant bit
    (c << (8+j)) & 0x8000, as int32 SWAR on DVE (bitvec ops are DVE-only,
    32-bit only); the b-reduction add tree is split: first level (h1) on
    DVE, lower levels (h2, w) on GPSIMD -- except the last chunk's h2/w,
    which stay on DVE so the tail does not pay the GPSIMD hand-off lag;
  - outT[f_shard, n_shard] = W'.T @ xT on the PE in bf16, accumulating
    the 8 contraction chunks across 8 concurrent PSUM banks (f32),
    W-chunk-outer so the PE consumes each chunk as it is decoded; the PE
    p-state ramp (0.65 -> 2.4 GHz after ~3us of busy) is pre-paid with a
    few throwaway matmuls on zeroed tiles while the decode warms up;
  - PSUM evacuation fuses the bias add (per-partition scalar, since the
    PSUM partition dim is f): Activation engine activation(Identity,
    bias=...), DVE / GPSIMD tensor_scalar(add); the 8 evacuations are
    spread across those three engines so the post-last-matmul tail is
    short; output DMAs are spread over the sync/scalar/vector queues.
"""

import numpy as np

try:
    import concourse.bass as bass  # noqa: F401
except ImportError:
    import sys

    sys.path.insert(0, "/opt/trn_rl_repo")
    import concourse.bass as bass  # noqa: F401

import concourse.bacc as bacc
import concourse.mybir as mybir
import concourse.tile as tile

B, T, NX, NF = 2, 2048, 1024, 1024
N_TOK = B * T
BITS = 8
G = NX // 8  # 128 code groups
PT, PF = 2, 4  # token-parallel x feature-parallel
TOK = N_TOK // PT  # tokens per core
NFS = NF // PF  # output features per core
P = 128
MM_N = 512  # moving free dim per matmul

AX = mybir.AxisListType
OP = mybir.AluOpType
F32 = mybir.dt.float32
BF16 = mybir.dt.float16  # compute dtype (fp16: same SWAR, more mantissa)
I16 = mybir.dt.int16
I32 = mybir.dt.int32
ACT_F = mybir.ActivationFunctionType
BF16NP = np.float16

MSK = -2147450880  # 0x80008000 as int32

# evacuation engine per PSUM group k = fb*nch + ch (see build_graph);
# GPSIMD cannot read PSUM, so only Act and DVE evacuate
EVAC_ENG = ["act", "vec", "act", "vec", "act", "vec", "act", "vec"]


def build_graph(nc, tok=TOK, nfs=NFS, pair_groups=None):
    del pair_groups  # legacy arg kept for test harness compatibility
    nfb = nfs // P  # f blocks of 128
    nch = tok // MM_N  # moving chunks
    xt_d = nc.dram_tensor("xt", (8, P, tok), BF16, kind="ExternalInput")
    cd_d = nc.dram_tensor("codes", (P, 8 * nfs), I16, kind="ExternalInput")
    sc_d = nc.dram_tensor("scales", (P, 8 * nfs), BF16, kind="ExternalInput")
    bi_d = nc.dram_tensor("biasv", (P, nfb), F32, kind="ExternalInput")
    out_d = nc.dram_tensor("out", (nfb, P, tok), BF16, kind="ExternalOutput")

    with tile.TileContext(nc) as tc:
        with (
            tc.tile_pool(name="xp", bufs=8) as xp,
            tc.tile_pool(name="cp", bufs=8) as cp,
            tc.tile_pool(name="wp", bufs=8) as wp,
            tc.tile_pool(name="sgp", bufs=2) as sgp,
            tc.tile_pool(name="wsgp", bufs=2) as wsgp,
            tc.tile_pool(name="qp", bufs=4) as qp,
            tc.tile_pool(name="cst", bufs=1) as cst,
            tc.tile_pool(name="op", bufs=8) as op_,
            tc.tile_pool(name="pp", bufs=8, space="PSUM") as pp,
        ):
            # --- loads; codes first: decode is the critical path ---
            cd = cp.tile([P, 8 * nfs], I16, tag="cd")
            nc.sync.dma_start(cd[:], cd_d[:])
            sc_bc = cst.tile([P, 8 * nfs], BF16, tag="sc_bc")
            nc.sync.dma_start(sc_bc[:], sc_d[:])
            bi_t = cst.tile([P, nfb], F32, tag="bi_t")
            nc.sync.dma_start(bi_t[:], bi_d[:])
            xts = []
            for j in range(8):
                xt = xp.tile([P, tok], BF16, tag="xt")
                nc.sync.dma_start(xt[:], xt_d[j])
                xts.append(xt)

            # --- Activation table prime + PE p-state warmup fodder ---
            prime = cst.tile([P, 1], BF16, tag="prime")
            nc.scalar.activation(
                prime[:], bi_t[:, 0:1], ACT_F.Identity, bias=bi_t[:, 0:1]
            )
            wz = cst.tile([P, P], BF16, tag="wz")
            nc.gpsimd.memset(wz[:], 0.0)
            xz = cst.tile([P, MM_N], BF16, tag="xz")
            nc.gpsimd.memset(xz[:], 0.0)

            pss = {}
            for fb in range(nfb):
                for ch in range(nch):
                    pss[(fb, ch)] = pp.tile(
                        [P, MM_N], F32, tag="ps", name=f"ps{fb}_{ch}"
                    )
            # Throwaway matmuls: ramp the PE to full clock while the DVE
            # decode runs, and keep the busy streak alive until the first
            # real chunk lands (idle gaps reset the p-state ramp); the real
            # j=0 matmul restarts its bank (start=True).
            for _ in range(20):
                nc.tensor.matmul(pss[(0, 0)][:], wz[:], xz[:], start=True, stop=False)

            # --- decode W chunks ---
            # Sign-bit trick: masked quant bit (inverted) XORed onto the
            # bf16 scale's sign gives +-scale exactly.  Bit ops run as int32
            # SWAR over int16-lane pairs; the bit inversion is folded into a
            # one-time sign-flip of the scale tile (host pre-negates):
            #   ((c << (8+j)) & M) ^ (sc ^ M)  ==  ((~c << (8+j)) & M) ^ sc
            sgs, wsgs, h1s, ws = [], [], [], []

            def emit_tsp(j):
                sg = sgp.tile([P, 8 * nfs], I16, tag="sg", name=f"sg{j}")
                nc.vector.tensor_scalar(
                    sg[:].bitcast(I32), cd[:].bitcast(I32), 8 + j, MSK,
                    OP.logical_shift_left, OP.bitwise_and,
                )
                sgs.append(sg)

            def emit_xor_h1(j):
                wsg = wsgp.tile([P, 8 * nfs], I16, tag="wsg", name=f"wsg{j}")
                nc.vector.tensor_tensor(
                    wsg[:].bitcast(I32), sgs[j][:].bitcast(I32),
                    sc_bc[:].bitcast(I32), OP.bitwise_xor,
                )
                wsgs.append(wsg)
                wv = wsg[:].bitcast(BF16)
                h1 = qp.tile([P, 4 * nfs], BF16, tag="h1", name=f"h1_{j}")
                nc.vector.tensor_tensor(
                    h1[:], wv[:, : 4 * nfs], wv[:, 4 * nfs :], OP.add
                )
                h1s.append(h1)

            def emit_h2w(j, eng):
                h2 = qp.tile([P, 2 * nfs], BF16, tag="h2", name=f"h2_{j}")
                eng.tensor_tensor(
                    h2[:], h1s[j][:, : 2 * nfs], h1s[j][:, 2 * nfs :], OP.add
                )
                w = wp.tile([P, nfs], BF16, tag="w", name=f"w{j}")
                eng.tensor_tensor(w[:], h2[:, :nfs], h2[:, nfs:], OP.add)
                ws.append(w)

            # DVE order: two TSPs up front to hide the scales-DMA wait,
            # then steady-state xor+h1 with the next TSP slotted between.
            emit_tsp(0)
            emit_tsp(1)
            emit_xor_h1(0)
            for j in range(1, 8):
                if j + 1 < 8:
                    emit_tsp(j + 1)
                emit_xor_h1(j)
            # gpsimd consumes h1_j for j=0..6; the last chunk's h2/w stay on
            # DVE so W7 lands without the cross-engine hand-off lag.
            for j in range(6):
                emit_h2w(j, nc.gpsimd)
            # j6: h2 on gpsimd, final add on DVE (lands just after DVE's own
            # j7 chain, so the PE's last two chunks arrive ~back-to-back)
            h2_6 = qp.tile([P, 2 * nfs], BF16, tag="h2", name="h2_6")
            nc.gpsimd.tensor_tensor(
                h2_6[:], h1s[6][:, : 2 * nfs], h1s[6][:, 2 * nfs :], OP.add
            )
            w6 = wp.tile([P, nfs], BF16, tag="w", name="w6")
            ws.append(w6)
            emit_h2w(7, nc.vector)
            nc.vector.tensor_tensor(w6[:], h2_6[:, :nfs], h2_6[:, nfs:], OP.add)
            # Python emission order above != engine order; Tile serializes
            # per engine in emission order, so reorder: we emitted all DVE
            # decode first, then gpsimd h2/w -- gpsimd only depends on h1_j,
            # which lands early; fine.

            # --- matmul: outT[f, n] = sum_j W_j.T @ xT_j  (bias in evac) ---
            # Chunk consumption order matches W readiness: w7 (DVE) lands
            # just before w6 (gpsimd h2 + DVE final add).
            groups = [(fb, ch) for fb in range(nfb) for ch in range(nch)]
            j_order = [0, 1, 2, 3, 4, 5, 7, 6]
            ob2s = {}
            for fb in range(nfb):
                for cp in range(nch // 2):
                    ob2s[(fb, cp)] = op_.tile(
                        [P, 2 * MM_N], BF16, tag="ob", name=f"ob{fb}_{cp}"
                    )
            for ji, j in enumerate(j_order):
                last = ji == 7
                for k, (fb, ch) in enumerate(groups):
                    nc.tensor.matmul(
                        pss[(fb, ch)][:],
                        ws[j][:, fb * P : (fb + 1) * P],
                        xts[j][:, ch * MM_N : (ch + 1) * MM_N],
                        start=(ji == 0),
                        stop=last,
                    )
                    if last:
                        # evacuate with fused bias add; Act/DVE alternate so
                        # the tail pipeline drains in parallel; pairs of
                        # token-chunks share one SBUF tile so the output
                        # needs only 4 larger DMAs
                        ob2 = ob2s[(fb, ch // 2)]
                        dst = ob2[:, (ch % 2) * MM_N : (ch % 2 + 1) * MM_N]
                        bcol = bi_t[:, fb : fb + 1]
                        if EVAC_ENG[k] == "act":
                            nc.scalar.activation(
                                dst, pss[(fb, ch)][:], ACT_F.Identity,
                                bias=bcol, scale=1.0,
                            )
                        else:
                            nc.vector.tensor_scalar(
                                dst, pss[(fb, ch)][:], bcol, None, OP.add
                            )
            # Output DMAs, 4 x 256KB, split over the SP and Act HWDGE queues
            for (fb, cp), ob2 in sorted(ob2s.items()):
                q = nc.sync if cp == 0 else nc.scalar
                q.dma_start(
                    out_d[fb][:, cp * 2 * MM_N : (cp + 1) * 2 * MM_N], ob2[:]
                )
    nc.compile()
    return nc


_I_PERM = 8 * (np.arange(NX) % G) + np.arange(NX) // G  # i' -> i


def host_prep(x, binary, scale, bias):
    """Layout-only sharding (plus x's bf16 compute-precision cast).
    Returns in_maps for cores 0..7 (pt = c//PF, pf = c%PF)."""
    x2 = np.ascontiguousarray(x.reshape(N_TOK, NX).T)[_I_PERM]  # (NX, N)
    x2 = x2.astype(BF16NP)  # compute dtype
    binary16 = binary.astype(np.int16)  # lossless: codes are 0..255
    in_maps = []
    for c in range(8):
        pt, pf = c // PF, c % PF
        f0 = pf * NFS
        xs = np.ascontiguousarray(x2[:, pt * TOK : (pt + 1) * TOK]).reshape(
            8, P, TOK
        )
        cs = np.ascontiguousarray(
            binary16[:, :, f0 : f0 + NFS].transpose(1, 0, 2)
        ).reshape(P, 8 * NFS)
        ss = np.ascontiguousarray(
            np.broadcast_to(
                (-scale[:, f0 : f0 + NFS].astype(BF16NP)).reshape(1, 8 * NFS),
                (P, 8 * NFS),
            )
        )
        bs = np.ascontiguousarray(
            bias[f0 : f0 + NFS].astype(np.float32).reshape(NFS // P, P).T
        )
        in_maps.append({"xt": xs, "codes": cs, "scales": ss, "biasv": bs})
    return in_maps


def host_assemble(results):
    """results[c]["out"]: (NFB, 128, TOK) -> full (B, T, NF)."""
    outT = np.empty((NF, N_TOK), dtype=np.float32)
    for c in range(8):
        pt, pf = c // PF, c % PF
        o = np.asarray(results[c]["out"], dtype=np.float32).reshape(NFS, TOK)
        outT[pf * NFS : (pf + 1) * NFS, pt * TOK : (pt + 1) * TOK] = o
    return np.ascontiguousarray(outT.T).reshape(B, T, NF)


_NC_CACHE = {}


def _get_nc():
    if "nc" not in _NC_CACHE:
        nc = bacc.Bacc(None, target_bir_lowering=False)
        build_graph(nc)
        _NC_CACHE["nc"] = nc
    return _NC_CACHE["nc"]


def kernel(**inputs):
    from concourse.bass_utils import run_bass_kernel_spmd

    inputs = {k: np.asarray(v) for k, v in inputs.items()}
    in_maps = host_prep(
        inputs["x"], inputs["binary"], inputs["scale"], inputs["bias"]
    )
    res = run_bass_kernel_spmd(_get_nc(), in_maps, core_ids=list(range(8)))
    return host_assemble(res.results)
